# revision 26
# baseline (speedup 1.0000x reference)
"""v3: v2 + per-slice weight tiles in block-0 consumption order (first
matmul waits on 32KB, not 1MB), block-0 x split gpsimd/scalar, outputs
alternating sync/vector queues to drain the output backlog."""

import numpy as np
import ml_dtypes

import concourse.mybir as mybir
from concourse import bacc
from concourse.bass_utils import run_bass_kernel_spmd
from concourse.tile import TileContext

B, S, F = 64, 32768, 6
E, L = 16, 80
MAX_DELAY = 10

NCORES = 8
BPC = B // NCORES
Q = 48
KWIN = 128
NTILE = 6
TPS = 5
NS = 6
PADF = 39
NCOLB = (S + Q - 1) // Q
BLKN = 512
NBLK = 11
CPAD = NBLK * BLKN
CTOT = BPC * NCOLB
LASTN = CTOT - (NBLK - 1) * BLKN

BF16 = ml_dtypes.bfloat16
LAST_RESULT = None


def _tile_of(s: int, tl: int) -> int:
    return tl + (1 if s >= 3 else 0)


def _build_weights(templates: np.ndarray, onset_delays: np.ndarray) -> np.ndarray:
    d = np.round(np.clip(onset_delays, -MAX_DELAY, MAX_DELAY)).astype(np.int64)
    idx = np.arange(L)
    src = idx[None, None, :] - d[:, :, None]
    valid = (src >= 0) & (src < L)
    delayed = np.take_along_axis(templates, np.clip(src, 0, L - 1), axis=2)
    delayed = np.where(valid, delayed, 0.0).astype(np.float32) / float(L * F)

    W = np.zeros((KWIN, NS, TPS, 128), dtype=np.float32)
    dd = np.arange(8)
    for s in range(NS):
        for tl in range(TPS):
            slot = 128 * _tile_of(s, tl) + np.arange(128)
            k = slot // F
            f = slot % F
            l = (8 * s + dd)[None, :] + 79 - k[:, None]
            ok = (l >= 0) & (l < L)
            g = delayed[:, f[:, None], np.clip(l, 0, L - 1)]
            g = np.where(ok[None], g, 0.0)
            W[:, s, tl, :] = g.transpose(1, 2, 0).reshape(128, 128)
    return np.ascontiguousarray(W.astype(BF16))


def _build_xsc(x: np.ndarray) -> np.ndarray:
    need = Q * (NCOLB - 1) + KWIN
    xpad = np.zeros((B, PADF + need, F), dtype=BF16)
    xpad[:, PADF:PADF + S, :] = x.astype(BF16)
    xflat = np.ascontiguousarray(xpad.reshape(B, -1))
    ez = xflat.strides[1]
    v = np.lib.stride_tricks.as_strided(
        xflat, shape=(B, NTILE, KWIN, NCOLB),
        strides=(xflat.strides[0], 128 * ez, ez, Q * F * ez),
    )
    out = np.zeros((NCORES, NTILE, KWIN, CPAD), dtype=BF16)
    for b in range(B):
        core, i = divmod(b, BPC)
        out[core, :, :, i * NCOLB:(i + 1) * NCOLB] = v[b]
    return out


def _build_program():
    f32 = mybir.dt.float32
    bf16 = mybir.dt.bfloat16
    nc = bacc.Bacc("TRN2", target_bir_lowering=False, debug=False)
    xsc = nc.dram_tensor("xsc", [NTILE, KWIN, CPAD], bf16, kind="ExternalInput")
    w = nc.dram_tensor("w", [KWIN, NS, TPS, 128], bf16, kind="ExternalInput")
    osc = nc.dram_tensor("osc", [NBLK, NS, 128, BLKN], bf16, kind="ExternalOutput")

    with TileContext(nc) as tc:
        with (
            tc.tile_pool(name="wp", bufs=1) as wp,
            tc.tile_pool(name="xp", bufs=24) as xp,
            tc.tile_pool(name="pp", bufs=8, space="PSUM") as pp,
            tc.tile_pool(name="op", bufs=6) as op,
        ):
            wr = w.rearrange("p s t m -> p (s t m)")
            wts = [None] * (NS * TPS)
            for t in range(NTILE):
                for s in range(NS):
                    tl = t - (1 if s >= 3 else 0)
                    if 0 <= tl < TPS:
                        wt1 = wp.tile([KWIN, 128], bf16, tag=f"w{s}_{tl}")
                        sl = slice((s * TPS + tl) * 128, (s * TPS + tl + 1) * 128)
                        nc.sync.dma_start(out=wt1, in_=wr[:, sl])
                        wts[s * TPS + tl] = wt1

            for blk in range(NBLK):
                n = BLKN if blk < NBLK - 1 else LASTN
                xtp = []
                for t in range(NTILE):
                    xf = xp.tile([KWIN, n], bf16, tag="xtp")
                    eng = nc.scalar if (blk == 0 and t >= 3) else nc.gpsimd
                    eng.dma_start(
                        out=xf, in_=xsc[t, :, blk * BLKN:blk * BLKN + n]
                    )
                    xtp.append(xf)
                pss = [
                    pp.tile([128, n], f32, tag="ps", name=f"ps_{blk}_{s}")
                    for s in range(NS)
                ]

                def evac(s, n=n, blk=blk, pss=pss):
                    ot = op.tile([128, n], bf16, tag="ot", name=f"ot_{blk}_{s}")
                    nc.vector.tensor_copy(out=ot, in_=pss[s])
                    q = nc.sync if (blk + s) % 2 == 0 else nc.scalar
                    q.dma_start(out=osc[blk, s, :, 0:n], in_=ot)

                if blk == 0:
                    for t in range(NTILE):
                        for s in range(NS):
                            tl = t - (1 if s >= 3 else 0)
                            if 0 <= tl < TPS:
                                nc.tensor.matmul(
                                    pss[s],
                                    wts[s * TPS + tl],
                                    xtp[t],
                                    start=(tl == 0),
                                    stop=(tl == TPS - 1),
                                    skip_group_check=True,
                                )
                    for s in range(NS):
                        evac(s)
                else:
                    for s in range(NS):
                        g = 1 if s >= 3 else 0
                        for tl in range(TPS):
                            nc.tensor.matmul(
                                pss[s],
                                wts[s * TPS + tl],
                                xtp[tl + g],
                                start=(tl == 0),
                                stop=(tl == TPS - 1),
                            )
                        evac(s)
    nc.compile()
    return nc


def kernel(x: np.ndarray, templates: np.ndarray, onset_delays: np.ndarray) -> np.ndarray:
    global LAST_RESULT
    x = np.ascontiguousarray(x, dtype=np.float32)
    templates = np.asarray(templates, dtype=np.float32)
    onset_delays = np.asarray(onset_delays, dtype=np.float32)

    W = _build_weights(templates, onset_delays)
    Xsc = _build_xsc(x)

    nc = _build_program()
    in_maps = [{"xsc": Xsc[c], "w": W} for c in range(NCORES)]
    res = run_bass_kernel_spmd(nc, in_maps, core_ids=list(range(NCORES)))
    LAST_RESULT = res

    osc = np.stack([np.asarray(r["osc"]) for r in res.results], axis=0)
    o = osc.astype(np.float32)
    o = o.reshape(NCORES, NBLK, NS, 8, E, BLKN)
    o = o.transpose(0, 1, 5, 2, 3, 4)
    o = np.ascontiguousarray(o).reshape(NCORES, CPAD, NS * 8 * E)
    o = o[:, :BPC * NCOLB, :].reshape(NCORES, BPC, NCOLB, NS, 8, E)
    o = o.reshape(B, NCOLB * Q, E)[:, :S, :]
    o = np.ascontiguousarray(o)
    o[:, S - 1, :] = 0.0
    return o
